# revision 3
# baseline (speedup 1.0000x reference)
"""Trainium2 Bass kernel for a cross-attention layer (v2).

Reference computation (per batch b):
    vision = inputs[b, :, :1024]; text = inputs[b, :, 1024:]
    Q = vision @ Wq.T + bq;  K = text @ Wk.T + bk;  V = text @ Wv.T + bv
    attn = softmax(Q @ K.T / 32, axis=-1)                 # [S, S]
    cav  = attn @ V                                       # [S, 1024]
    cat  = attn.T @ vision                                # [S, 1024]

Sharding: 8 cores = 4 batches x 2 query-halves (1024 q rows each).
cc mode: each core projects K/V only for its OWN 1024-key half; the
core pair exchanges KT then V via two pairwise AllGathers (the KT
exchange overlaps the V projection; V exchange overlaps Q) and reads
both halves back from the gathered buffers (parity-independent
layout: block 0 = even core's half = keys [0,1024)). Non-cc mode
projects both halves locally (+55us PE, no collective).

NOTE: a collective inside a For_i hardware loop wedges the device
(NRT requires straight-line collective ordering), so reps>1 timing
builds replace the AllGathers with same-sized local DRAM copies
(cc_mode="fake"); the graded reps=1 path uses the real collective.

Per-core algorithm (all SBUF-resident, no DRAM spill):
  A:  KT[e,k] halves = proj; exchange. V[k,e] halves = proj; exchange.
  0:  QT[e,q] = Wq @ visionT + bq
  1a: per q-tile: exp_s[q, 0:1024] = exp(QT.T @ KT0 / 32) (+Z part),
      PE-transpose 128x128 blocks -> expT0[k,q]
  1b: per q-tile: same for half 1, then cav[q,:] accumulated over the
      FULL key range in one PSUM group (expT as lhsT, V as rhs),
      scaled by 1/Z directly out of PSUM (ACT per-partition scale)
      -> DMA out; vis_sc[q,:] = vision[q,:] * 1/Z (ACT) as Z finalizes.
  2:  per k-tile: cat[k,:] = sum_q exp_s[q,k] vis_sc[q,:] accumulated
      in PSUM over q-tiles -> DMA out.
cat is a partial (own q-half only); host sums the pair.

PSUM layout (8 banks): big 2x[128,1024] (projections, scores, cat;
4 banks) + cav 1x[128,1024] (2 banks) + tr 2x[128,128] (transposes).
SBUF: persistent ~115KB; "inp" pool (weights/activations, 80KB)
closes after phase 0 and its region is reused by the "attn" pool
(exp/expT/outputs, 80KB).
"""

import numpy as np
import ml_dtypes

B, S, D = 4, 2048, 1024
QH = 1024          # query rows per core
NCORES = 8

_CACHE = {}


def _build(reps=1, cc_mode="real"):
    """cc_mode: 'real' (AllGather; reps must be 1), 'fake' (local DRAM
    copy stand-in, safe under For_i), 'none' (project both halves
    locally)."""
    import contextlib

    import concourse.mybir as mybir
    from concourse import bacc
    from concourse.masks import make_identity
    from concourse.tile import TileContext

    assert cc_mode in ("real", "fake", "none")
    use_cc = cc_mode != "none"
    assert not (cc_mode == "real" and reps > 1), "collective can't loop"

    DT = mybir.dt.bfloat16
    F32 = mybir.dt.float32
    AF = mybir.ActivationFunctionType
    ADD = mybir.AluOpType.add
    SCALE = float(1.0 / np.sqrt(np.float32(D)))

    nc = bacc.Bacc()
    visionT = nc.dram_tensor("visionT", [D, QH], DT, kind="ExternalInput")
    vision = nc.dram_tensor("vision", [QH, D], DT, kind="ExternalInput")
    # own half of textT: columns [h*1024, (h+1)*1024) of the full [D, S]
    tw = S // 2 if use_cc else S
    textTo = nc.dram_tensor("textTo", [D, tw], DT, kind="ExternalInput")
    wqT = nc.dram_tensor("wqT", [D, D], DT, kind="ExternalInput")
    wkT = nc.dram_tensor("wkT", [D, D], DT, kind="ExternalInput")
    wvT = nc.dram_tensor("wvT", [D, D], DT, kind="ExternalInput")
    bqp = nc.dram_tensor("bqp", [128, 8], F32, kind="ExternalInput")
    bkp = nc.dram_tensor("bkp", [128, 8], F32, kind="ExternalInput")
    bvr = nc.dram_tensor("bvr", [1, D], DT, kind="ExternalInput")
    cav_o = nc.dram_tensor("cav", [QH, D], F32, kind="ExternalOutput")
    cat_o = nc.dram_tensor("catp", [S, D], F32, kind="ExternalOutput")
    if use_cc:
        kt_own_d = nc.dram_tensor("kt_own_d", [D, D], DT)
        v_own_d = nc.dram_tensor("v_own_d", [D, D], DT)
        kt_sh_d = nc.dram_tensor("kt_sh_d", [2, D, D], DT)
        v_sh_d = nc.dram_tensor("v_sh_d", [2, D, D], DT)

    visionT_r = visionT.rearrange("(dt p) q -> p dt q", p=128)
    vision_r = vision.rearrange("(qt p) d -> p qt d", p=128)
    textTo_r = textTo.rearrange("(dt p) k -> p dt k", p=128)
    wq_r = wqT.rearrange("(dt p) e -> p dt e", p=128)
    wk_r = wkT.rearrange("(dt p) e -> p dt e", p=128)
    wv_r = wvT.rearrange("(dt p) e -> p dt e", p=128)
    cav_r = cav_o.rearrange("(qt p) e -> p qt e", p=128)
    cat_r = cat_o.rearrange("(kt p) d -> p kt d", p=128)

    with TileContext(nc) as tc:
        rep_cm = tc.For_i(0, reps, 1) if reps > 1 else contextlib.nullcontext()
        with (
            rep_cm,
            tc.tile_pool(name="const", bufs=1) as const,
            tc.tile_pool(name="kv", bufs=1) as kvp,
            tc.tile_pool(name="stats", bufs=1) as stats,
            tc.tile_pool(name="bigps", bufs=2, space="PSUM") as bigps,
            tc.tile_pool(name="cavps", bufs=1, space="PSUM") as cavps,
            tc.tile_pool(name="trps", bufs=2, space="PSUM") as trps,
        ):
            bq_sb = const.tile([128, 8], F32)
            bk_sb = const.tile([128, 8], F32)
            bv_bc = const.tile([128, D], DT)
            ident = const.tile([128, 128], DT)

            qt_sb = const.tile([128, 8, QH], DT)
            vis_sb = const.tile([128, 8, D], DT)
            # K/V for both halves: [h][128, 8, 1024]
            kt_h = [kvp.tile([128, 8, D], DT, tag=f"kt{h}", name=f"kt{h}")
                    for h in range(2)]
            v_h = [kvp.tile([128, 8, D], DT, tag=f"v{h}", name=f"v{h}")
                   for h in range(2)]
            z_own = stats.tile([128, 8], F32)
            z_acc = stats.tile([128, 8], F32)
            invz = stats.tile([128, 8], F32)

            # ---- input loads + phases A/0 (weights freed after 0) ----
            with tc.tile_pool(name="inp", bufs=1) as inp:
                # startup-critical loads first, in per-dt chunks so the
                # first projection matmuls gate on 256KB, not 2MB.
                wk_sb = inp.tile([128, 8, D], DT)
                for dt in range(4):
                    nc.sync.dma_start(out=wk_sb[:, 2 * dt:2 * dt + 2, :],
                                      in_=wk_r[:, 2 * dt:2 * dt + 2, :])
                tTo_t = []
                for hh in range(1 if use_cc else 2):
                    tTo = inp.tile([128, 8, D], DT, tag=f"tTo{hh}",
                                   name=f"tTo{hh}")
                    for dt in range(4):
                        nc.sync.dma_start(
                            out=tTo[:, 2 * dt:2 * dt + 2, :],
                            in_=textTo_r[:, 2 * dt:2 * dt + 2,
                                         hh * D:(hh + 1) * D])
                    tTo_t.append(tTo)
                nc.sync.dma_start(out=bk_sb, in_=bkp[:])
                nc.sync.dma_start(out=bq_sb, in_=bqp[:])
                nc.sync.dma_start(out=bv_bc, in_=bvr[:].to_broadcast((128, D)))
                make_identity(nc, ident)
                wv_sb = inp.tile([128, 8, D], DT)
                nc.sync.dma_start(out=wv_sb, in_=wv_r)
                vT_sb = inp.tile([128, 8, QH], DT)
                nc.sync.dma_start(out=vT_sb, in_=visionT_r)
                wq_sb = inp.tile([128, 8, D], DT)
                nc.sync.dma_start(out=wq_sb, in_=wq_r)
                nc.sync.dma_start(out=vis_sb, in_=vision_r)

                def exchange(own_tile, own_d, sh_d, dest, rearr):
                    """DMA own half out, AllGather the pair, read both
                    halves back in absolute key order."""
                    nc.sync.dma_start(out=own_d[:].rearrange(rearr, p=128),
                                      in_=own_tile)
                    if cc_mode == "real":
                        nc.gpsimd.collective_compute(
                            "AllGather",
                            mybir.AluOpType.bypass,
                            replica_groups=[[2 * i, 2 * i + 1]
                                            for i in range(4)],
                            ins=[own_d[:]],
                            outs=[sh_d[:]],
                        )
                    else:  # timing stand-in, loop-safe
                        nc.sync.dma_start(out=sh_d[0], in_=own_d[:])
                        nc.sync.dma_start(out=sh_d[1], in_=own_d[:])
                    for h in range(2):
                        nc.sync.dma_start(
                            out=dest[h],
                            in_=sh_d[h].rearrange(rearr, p=128),
                        )

                # K projection (own half into kt_h[hh] as scratch)
                for hh in range(1 if use_cc else 2):
                    for et in range(8):
                        ps = bigps.tile([128, 1024], F32, tag="big")
                        for kc in range(2):
                            for dt in range(8):
                                nc.tensor.matmul(
                                    ps[:, kc * 512:(kc + 1) * 512],
                                    lhsT=wk_sb[:, dt, et * 128:(et + 1) * 128],
                                    rhs=tTo_t[hh][:, dt, kc * 512:(kc + 1) * 512],
                                    start=(dt == 0),
                                    stop=(dt == 7),
                                )
                        nc.scalar.activation(
                            out=kt_h[hh][:, et, :],
                            in_=ps,
                            func=AF.Identity,
                            bias=bk_sb[:, et:et + 1],
                            scale=1.0,
                        )
                if use_cc:
                    exchange(kt_h[0], kt_own_d, kt_sh_d, kt_h,
                             "(et p) k -> p et k")

                # V projection
                for hh in range(1 if use_cc else 2):
                    for kst in range(8):
                        ps = bigps.tile([128, 1024], F32, tag="big")
                        for ec in range(2):
                            for dt in range(8):
                                nc.tensor.matmul(
                                    ps[:, ec * 512:(ec + 1) * 512],
                                    lhsT=tTo_t[hh][:, dt, kst * 128:(kst + 1) * 128],
                                    rhs=wv_sb[:, dt, ec * 512:(ec + 1) * 512],
                                    start=(dt == 0),
                                    stop=(dt == 7),
                                )
                        nc.vector.tensor_tensor(
                            out=v_h[hh][:, kst, :],
                            in0=ps,
                            in1=bv_bc,
                            op=ADD,
                        )
                if use_cc:
                    exchange(v_h[0], v_own_d, v_sh_d, v_h,
                             "(kt p) e -> p kt e")

                # Phase 0: QT[e,q] = Wq @ visionT + bq
                for et in range(8):
                    ps = bigps.tile([128, 1024], F32, tag="big")
                    for qc in range(2):
                        for dt in range(8):
                            nc.tensor.matmul(
                                ps[:, qc * 512:(qc + 1) * 512],
                                lhsT=wq_sb[:, dt, et * 128:(et + 1) * 128],
                                rhs=vT_sb[:, dt, qc * 512:(qc + 1) * 512],
                                start=(dt == 0),
                                stop=(dt == 7),
                            )
                    nc.scalar.activation(
                        out=qt_sb[:, et, :],
                        in_=ps,
                        func=AF.Identity,
                        bias=bq_sb[:, et:et + 1],
                        scale=1.0,
                    )

            # ---- Phases 1a/1b/2: attn pool reuses the inp region ----
            with tc.tile_pool(name="attn", bufs=1) as attn:
                # exp_s resident [128, qt, 2048]; expT per half
                exps = attn.tile([128, 8, S], DT, tag="exps")
                expt_h = [attn.tile([128, 8, QH], DT, tag=f"expt{h}",
                                    name=f"expt{h}") for h in range(2)]
                vis_sc = attn.tile([128, 8, D], DT)

                def scores_qt(h, qt):
                    """exp_s[:, qt, h*1024:...] and expT for half h."""
                    ps = bigps.tile([128, 1024], F32, tag="big")
                    for kc in range(2):
                        for et in range(8):
                            nc.tensor.matmul(
                                ps[:, kc * 512:(kc + 1) * 512],
                                lhsT=qt_sb[:, et, qt * 128:(qt + 1) * 128],
                                rhs=kt_h[h][:, et, kc * 512:(kc + 1) * 512],
                                start=(et == 0),
                                stop=(et == 7),
                            )
                    zp = attn.tile([128, 1], F32, tag="zp", bufs=4)
                    nc.scalar.activation(
                        out=exps[:, qt, h * 1024:(h + 1) * 1024],
                        in_=ps,
                        func=AF.Exp,
                        scale=SCALE,
                        accum_out=zp,
                    )
                    if h == 0:
                        nc.vector.tensor_copy(out=z_own[:, qt:qt + 1], in_=zp)
                    else:
                        nc.vector.tensor_add(
                            out=z_acc[:, qt:qt + 1],
                            in0=z_own[:, qt:qt + 1],
                            in1=zp,
                        )
                    for kst in range(8):
                        pst = trps.tile([128, 128], DT, tag="tr")
                        nc.tensor.transpose(
                            out=pst,
                            in_=exps[:, qt, h * 1024 + kst * 128:
                                     h * 1024 + (kst + 1) * 128],
                            identity=ident,
                        )
                        nc.vector.tensor_copy(
                            out=expt_h[h][:, kst, qt * 128:(qt + 1) * 128],
                            in_=pst,
                        )

                # 1a: own-half (gated only on the KT readback)
                for qt in range(8):
                    scores_qt(0, qt)

                # 1b: pair half + full-k cav in PSUM
                for qt in range(8):
                    scores_qt(1, qt)
                    cps = cavps.tile([128, 1024], F32, tag="cav")
                    for ec in range(2):
                        for h2 in range(2):
                            for kst in range(8):
                                nc.tensor.matmul(
                                    cps[:, ec * 512:(ec + 1) * 512],
                                    lhsT=expt_h[h2][:, kst,
                                                    qt * 128:(qt + 1) * 128],
                                    rhs=v_h[h2][:, kst,
                                                ec * 512:(ec + 1) * 512],
                                    start=(h2 == 0 and kst == 0),
                                    stop=(h2 == 1 and kst == 7),
                                )
                    nc.vector.reciprocal(
                        out=invz[:, qt:qt + 1], in_=z_acc[:, qt:qt + 1]
                    )
                    cav_out = attn.tile([128, D], F32, tag="cavo", bufs=2)
                    nc.scalar.activation(
                        out=cav_out,
                        in_=cps,
                        func=AF.Copy,
                        scale=invz[:, qt:qt + 1],
                    )
                    nc.sync.dma_start(out=cav_r[:, qt, :], in_=cav_out)
                    nc.scalar.activation(
                        out=vis_sc[:, qt, :],
                        in_=vis_sb[:, qt, :],
                        func=AF.Copy,
                        scale=invz[:, qt:qt + 1],
                    )

                # 2: cat[k,d] = sum_q exp_s[q,k] vis_sc[q,d]
                for kk in range(16):
                    ps = bigps.tile([128, 1024], F32, tag="big")
                    for dc in range(2):
                        for qt in range(8):
                            nc.tensor.matmul(
                                ps[:, dc * 512:(dc + 1) * 512],
                                lhsT=exps[:, qt, kk * 128:(kk + 1) * 128],
                                rhs=vis_sc[:, qt, dc * 512:(dc + 1) * 512],
                                start=(qt == 0),
                                stop=(qt == 7),
                            )
                    cat_sb = attn.tile([128, D], F32, tag="cato", bufs=2)
                    nc.vector.tensor_copy(out=cat_sb, in_=ps)
                    nc.sync.dma_start(out=cat_r[:, kk, :], in_=cat_sb)
    nc.compile()
    return nc


def _get_nc(reps=1, cc_mode="real"):
    key = ("nc", reps, cc_mode)
    if key not in _CACHE:
        _CACHE[key] = _build(reps, cc_mode)
    return _CACHE[key]


def _prep_in_maps(inputs, Wq, bq, Wk, bk, Wv, bv, use_cc=True):
    bf = ml_dtypes.bfloat16
    x = np.asarray(inputs, np.float32)
    wqT = np.ascontiguousarray(np.asarray(Wq, np.float32).T.astype(bf))
    wkT = np.ascontiguousarray(np.asarray(Wk, np.float32).T.astype(bf))
    wvT = np.ascontiguousarray(np.asarray(Wv, np.float32).T.astype(bf))
    bqp = np.ascontiguousarray(np.asarray(bq, np.float32).reshape(8, 128).T)
    bkp = np.ascontiguousarray(np.asarray(bk, np.float32).reshape(8, 128).T)
    bvr = np.asarray(bv, np.float32).astype(bf).reshape(1, D)
    in_maps = []
    for c in range(NCORES):
        b, h = divmod(c, 2)
        vis = x[b, :, :D]
        txt = x[b, :, D:]
        visc = vis[h * QH:(h + 1) * QH]
        in_maps.append({
            "visionT": np.ascontiguousarray(visc.T.astype(bf)),
            "vision": np.ascontiguousarray(visc.astype(bf)),
            "textTo": np.ascontiguousarray(
                (txt[h * QH:(h + 1) * QH] if use_cc else txt).T.astype(bf)
            ),
            "wqT": wqT, "wkT": wkT, "wvT": wvT,
            "bqp": bqp, "bkp": bkp, "bvr": bvr,
        })
    return in_maps


def run_on_device(in_maps, trace=False, reps=1, cc_mode="real"):
    from concourse.bass_utils import run_bass_kernel_spmd

    nc = _get_nc(reps, cc_mode)
    return run_bass_kernel_spmd(
        nc, in_maps, core_ids=list(range(NCORES)), trace=trace
    )


def _gather(results):
    cav_full = np.empty((B, S, D), np.float32)
    cat_full = np.zeros((B, S, D), np.float32)
    for c in range(NCORES):
        b, h = divmod(c, 2)
        cav_full[b, h * QH:(h + 1) * QH] = results[c]["cav"]
        cat_full[b] += results[c]["catp"]
    return cav_full, cat_full


CC_MODE = "real"


def kernel(**inputs):
    # Try the collective build twice (transient axon/NRT hiccups happen),
    # then fall back to the collective-free build, which is ~20% slower
    # but numerically identical and immune to collective flakiness.
    last_err = None
    for cc_mode in (CC_MODE, CC_MODE, "none"):
        in_maps = _prep_in_maps(**inputs, use_cc=(cc_mode != "none"))
        try:
            res = run_on_device(in_maps, trace=False, cc_mode=cc_mode)
            return _gather(res.results)
        except Exception as e:
            last_err = e
    raise last_err
